# revision 20
# baseline (speedup 1.0000x reference)
"""AdaptivePatchEmbedding (MoE routing) Trainium2 Bass kernel.

Full inputs in, full output out. Shards the flattened B*C=1344 row axis
across 8 NeuronCores (168 rows each); weights are replicated. The host
pre-transposes each core's x shard to xt [32 values, rows*32 regions] so no
on-chip transposes are needed. Per core the kernel processes 42 blocks of
128 regions:
  - classifier: h = relu(w1 @ x_region + b1) as [64,128], logits as
    [3,128] with the b2 bias applied by the scalar engine on eviction
  - first-max argmax -> one-hot mask row [1, 3*128] on DVE, broadcast to
    32 partitions with a single K=1 ones-matmul
  - mask-scale xT per expert along the free dim, then per output slot
    accumulate the 3 mask-weighted expert matmuls (K=32, zero-padded
    weights select the patch) in one PSUM bank
  - evict PSUM with a fused add of the sinusoidal positional encoding
  - contiguous 1 MB DMA store per block
"""

import math
import sys
from contextlib import ExitStack

import numpy as np

for _p in ("/opt/trn_rl_repo",):
    if _p not in sys.path:
        sys.path.insert(0, _p)

import concourse.bass as bass
import concourse.tile as tile
from concourse import bacc, mybir
from concourse.bass_utils import run_bass_kernel_spmd

F32 = mybir.dt.float32
ALU = mybir.AluOpType
ACTF = mybir.ActivationFunctionType

B, C, L = 64, 21, 1024
N = B * C              # 1344 rows
NCORES = 8
ROWS = N // NCORES     # 168 rows per core
R = 32                 # regions per row (L / 32)
TGT = 4                # output slots per region
D = 512                # d_model
RPB = 128              # regions per block
E1_PATCH = [0, 0, 0, 1]   # repeat_interleave(3)[:4] for n_p=2


def _sinusoidal_pe(T, DM):
    pe = np.zeros((T, DM), np.float32)
    pos = np.arange(T, dtype=np.float32)[:, None]
    div = np.exp(np.arange(0, DM, 2, dtype=np.float32) * -(math.log(10000.0) / DM))
    pe[:, 0::2] = np.sin(pos * div)
    pe[:, 1::2] = np.cos(pos * div)
    return pe


def build(rows=ROWS):
    blocks = rows * R // RPB
    nregions = rows * R
    nc = bacc.Bacc(None, target_bir_lowering=False)

    xt_d = nc.declare_dram_parameter("xt", [32, nregions], F32, isOutput=False)
    w1t_d = nc.declare_dram_parameter("w1t", [32, 64], F32, isOutput=False)
    b1_d = nc.declare_dram_parameter("b1c", [64, 1], F32, isOutput=False)
    w2t_d = nc.declare_dram_parameter("w2t", [64, 3], F32, isOutput=False)
    b2r_d = nc.declare_dram_parameter("b2r", [1, 3 * RPB], F32, isOutput=False)
    on_d = nc.declare_dram_parameter("ones", [1, 32], F32, isOutput=False)
    # expert weights zero-padded to K=32; free block (e*TGT+t)*D holds the
    # weights of expert e routing to output slot t.
    wall_d = nc.declare_dram_parameter("wall", [32, 3 * TGT * D], F32,
                                       isOutput=False)
    pe_d = nc.declare_dram_parameter("pe", [RPB, TGT * D], F32, isOutput=False)
    out_d = nc.declare_dram_parameter("out", [rows, R * TGT, D], F32,
                                      isOutput=True)

    with tile.TileContext(nc) as tc, ExitStack() as ctx:
        const = ctx.enter_context(tc.tile_pool(name="const", bufs=1))
        xin = ctx.enter_context(tc.tile_pool(name="xin", bufs=4))
        work = ctx.enter_context(tc.tile_pool(name="work", bufs=3))
        outp = ctx.enter_context(tc.tile_pool(name="outp", bufs=3))
        ps_h = ctx.enter_context(
            tc.tile_pool(name="ps_h", bufs=1, space=bass.MemorySpace.PSUM))
        ps_l = ctx.enter_context(
            tc.tile_pool(name="ps_l", bufs=1, space=bass.MemorySpace.PSUM))
        ps_m = ctx.enter_context(
            tc.tile_pool(name="ps_m", bufs=2, space=bass.MemorySpace.PSUM))
        ps_e = ctx.enter_context(
            tc.tile_pool(name="ps_e", bufs=4, space=bass.MemorySpace.PSUM))

        w1t = const.tile([32, 64], F32)
        nc.sync.dma_start(w1t[:], w1t_d[:, :])
        b1 = const.tile([64, 1], F32)
        nc.sync.dma_start(b1[:], b1_d[:, :])
        w2t = const.tile([64, 3], F32)
        nc.sync.dma_start(w2t[:], w2t_d[:, :])
        b2r = const.tile([1, 3 * RPB], F32)
        nc.sync.dma_start(b2r[:], b2r_d[:, :])
        ones = const.tile([1, 32], F32)
        nc.sync.dma_start(ones[:], on_d[:, :])
        wall = const.tile([32, 3 * TGT * D], F32)
        nc.sync.dma_start(wall[:], wall_d[:, :])
        pe = const.tile([RPB, TGT * D], F32)
        nc.sync.dma_start(pe[:], pe_d[:, :])

        ov = out_d[:, :, :].rearrange("n (r t) d -> (n r) (t d)", t=TGT)

        for b in range(blocks):
            xT = xin.tile([32, RPB], F32, tag="xT")
            nc.sync.dma_start(xT[:], xt_d[:, RPB * b:RPB * (b + 1)])

            # classifier: h = relu(w1 @ x_region + b1)  [64, 128]
            hp = ps_h.tile([64, RPB], F32, tag="hp")
            nc.tensor.matmul(hp[:], w1t[:], xT[:], start=True, stop=True)
            ha = work.tile([64, RPB], F32, tag="ha")
            nc.scalar.activation(ha[:], hp[:], ACTF.Relu, bias=b1[:])

            # logits as one row [1, 3*128]: three M=1 matmuls, one per class
            lp = ps_l.tile([1, 3 * RPB], F32, tag="lp")
            for e in range(3):
                nc.tensor.matmul(lp[:, RPB * e:RPB * (e + 1)],
                                 w2t[:, e:e + 1], ha[:], start=True, stop=True)
            lg = work.tile([1, 3 * RPB], F32, tag="lg")
            nc.vector.tensor_tensor(lg[:], lp[:], b2r[:], ALU.add)
            l0, l1, l2 = (lg[:, RPB * e:RPB * (e + 1)] for e in range(3))

            # first-max argmax -> one-hot mask row [1, 3*128]
            ma = work.tile([1, 3 * RPB], F32, tag="ma")
            c0 = work.tile([1, RPB], F32, tag="c0")
            c1 = work.tile([1, RPB], F32, tag="c1")
            nc.vector.tensor_tensor(c0[:], l0, l1, ALU.is_ge)
            nc.vector.tensor_tensor(c1[:], l0, l2, ALU.is_ge)
            nc.vector.tensor_tensor(ma[:, 0:RPB], c0[:], c1[:], ALU.mult)
            nc.vector.tensor_tensor(c0[:], l1, l0, ALU.is_gt)
            nc.vector.tensor_tensor(c1[:], l1, l2, ALU.is_ge)
            nc.vector.tensor_tensor(ma[:, RPB:2 * RPB], c0[:], c1[:], ALU.mult)
            nc.vector.tensor_tensor(c0[:], ma[:, 0:RPB], ma[:, RPB:2 * RPB],
                                    ALU.add)
            nc.vector.tensor_scalar(ma[:, 2 * RPB:3 * RPB], c0[:], -1.0, 1.0,
                                    ALU.mult, ALU.add)

            # broadcast masks to 32 partitions (K=1 matmul), apply to xT
            mb = ps_m.tile([32, 3 * RPB], F32, tag="mb")
            nc.tensor.matmul(mb[:], ones[:], ma[:], start=True, stop=True)
            xm3 = work.tile([32, 3 * RPB], F32, tag="xm3")
            for e in range(3):
                nc.vector.tensor_tensor(xm3[:, RPB * e:RPB * (e + 1)], xT[:],
                                        mb[:, RPB * e:RPB * (e + 1)], ALU.mult)

            ot = outp.tile([RPB, TGT * D], F32, tag="ot")
            for t in range(TGT):
                ps = ps_e.tile([RPB, D], F32, tag="ps")
                nc.tensor.matmul(ps[:], xm3[:, 0:RPB],
                                 wall[:, D * t:D * (t + 1)],
                                 start=True, stop=False)
                nc.tensor.matmul(ps[:], xm3[:, RPB:2 * RPB],
                                 wall[:, D * (TGT + t):D * (TGT + t + 1)],
                                 start=False, stop=False)
                nc.tensor.matmul(ps[:], xm3[:, 2 * RPB:3 * RPB],
                                 wall[:, D * (2 * TGT + t):D * (2 * TGT + t + 1)],
                                 start=False, stop=True)
                nc.vector.tensor_tensor(ot[:, D * t:D * (t + 1)], ps[:],
                                        pe[:, D * t:D * (t + 1)], ALU.add)

            nc.sync.dma_start(ov[RPB * b:RPB * (b + 1), :], ot[:])

    nc.compile()
    return nc


def _build_wall(w_emb0, w_emb1, w_emb2):
    wall = np.zeros((32, 3 * TGT * D), np.float32)
    for t in range(TGT):
        wall[8 * t:8 * t + 8, D * t:D * (t + 1)] = w_emb0.T
        p1 = E1_PATCH[t]
        wall[16 * p1:16 * p1 + 16,
             D * (TGT + t):D * (TGT + t + 1)] = w_emb1.T
        wall[:, D * (2 * TGT + t):D * (2 * TGT + t + 1)] = w_emb2.T
    return wall


def _host_inputs(w1, b1, w2, b2, w_emb0, w_emb1, w_emb2):
    pe = _sinusoidal_pe(R * TGT, D)                          # [128, 512]
    pe_big = np.tile(pe.reshape(R, TGT * D), (RPB // R, 1))  # [128, 2048]
    return {
        "w1t": np.ascontiguousarray(w1.T),
        "b1c": np.ascontiguousarray(b1.reshape(64, 1)),
        "w2t": np.ascontiguousarray(w2.T),
        "b2r": np.ascontiguousarray(np.repeat(b2, RPB).reshape(1, 3 * RPB)),
        "ones": np.ones((1, 32), np.float32),
        "wall": _build_wall(w_emb0, w_emb1, w_emb2),
        "pe": np.ascontiguousarray(pe_big),
    }


_NC_CACHE = {}


def kernel(x, w1, b1, w2, b2, w_emb0, w_emb1, w_emb2, **run_kw):
    x = np.ascontiguousarray(np.asarray(x, np.float32)).reshape(N, L)
    shared = _host_inputs(
        np.asarray(w1, np.float32), np.asarray(b1, np.float32),
        np.asarray(w2, np.float32), np.asarray(b2, np.float32),
        np.asarray(w_emb0, np.float32), np.asarray(w_emb1, np.float32),
        np.asarray(w_emb2, np.float32))

    if "nc" not in _NC_CACHE:
        _NC_CACHE["nc"] = build()
    nc = _NC_CACHE["nc"]

    in_maps = []
    for i in range(NCORES):
        m = dict(shared)
        shard = x[i * ROWS:(i + 1) * ROWS]
        m["xt"] = np.ascontiguousarray(shard.reshape(ROWS * R, 32).T)
        in_maps.append(m)

    res = run_bass_kernel_spmd(nc, in_maps, list(range(NCORES)), **run_kw)
    full = np.concatenate([res.results[i]["out"] for i in range(NCORES)],
                          axis=0)
    kernel.last_result = res
    return full, C


if __name__ == "__main__":
    print("smoke build only")
    build(rows=8)
    print("ok")


# revision 24
# speedup vs baseline: 3.9692x; 3.9692x over previous
"""AdaptivePatchEmbedding (MoE routing) Trainium2 Bass kernel.

Full inputs in, full output out. Shards the flattened B*C=1344 row axis
across 8 NeuronCores (168 rows each); weights are replicated. The host
pre-transposes each core's x shard to xt [32 values, rows*32 regions] in
bf16 (fp32 matmuls stream at half rate and the PE clock is capped at
1.2 GHz here, so columns are precious; PSUM still accumulates fp32).
Per core the kernel processes 42 blocks of 128 regions:
  - xm [128, 128]: rows 0:96 = x replicated per expert (DMA x3), rows
    96:128 = a one-hot region-selector constant
  - classifier h = relu(w1 @ x + b1) via zero-padded w1 (full-tile
    operands only: matmuls with partition-base > 0 operands crash the
    runtime); logit differences D01/D02 via one N=2 matmul
  - first-max argmax -> one-hot masks ma3 [128reg, 3] on DVE (m1 stored
    negated, expert-1 weights negated to compensate), replicated to
    ma3r [128, 96] by a 0-stride DVE copy, transposed+broadcast to
    mb [96, 128] by one matmul against the identity
  - xmm: masked x (rows 0:96) + selector rows untouched; ONE K=128
    matmul per output slot then computes all three experts' masked
    embeddings AND adds the positional encoding (stored in the selector
    rows of the weight matrix) in a single pass: 4x512 PE columns/block
  - PSUM evicted by plain scalar-engine copies; contiguous 1 MB DMA out
"""

import math
import sys
from contextlib import ExitStack

import numpy as np

for _p in ("/opt/trn_rl_repo",):
    if _p not in sys.path:
        sys.path.insert(0, _p)

import ml_dtypes

import concourse.bass as bass
import concourse.tile as tile
from concourse import bacc, mybir
from concourse.bass_utils import run_bass_kernel_spmd

F32 = mybir.dt.float32
BF16 = mybir.dt.bfloat16
NPBF = ml_dtypes.bfloat16
ALU = mybir.AluOpType
ACTF = mybir.ActivationFunctionType

B, C, L = 64, 21, 1024
N = B * C              # 1344 rows
NCORES = 8
ROWS = N // NCORES     # 168 rows per core
R = 32                 # regions per row (L / 32)
TGT = 4                # output slots per region
D = 512                # d_model
RPB = 128              # regions per block
E1_PATCH = [0, 0, 0, 1]   # repeat_interleave(3)[:4] for n_p=2


def _sinusoidal_pe(T, DM):
    pe = np.zeros((T, DM), np.float32)
    pos = np.arange(T, dtype=np.float32)[:, None]
    div = np.exp(np.arange(0, DM, 2, dtype=np.float32) * -(math.log(10000.0) / DM))
    pe[:, 0::2] = np.sin(pos * div)
    pe[:, 1::2] = np.cos(pos * div)
    return pe


def build(rows=ROWS):
    blocks = rows * R // RPB
    nregions = rows * R
    nc = bacc.Bacc(None, target_bir_lowering=False)

    xt_d = nc.declare_dram_parameter("xt", [32, nregions], BF16, isOutput=False)
    w1t_d = nc.declare_dram_parameter("w1t", [96, 64], BF16, isOutput=False)
    b1_d = nc.declare_dram_parameter("b1c", [64, 1], F32, isOutput=False)
    w2d_d = nc.declare_dram_parameter("w2d", [64, 2], BF16, isOutput=False)
    b2d_d = nc.declare_dram_parameter("b2d", [RPB, 2], F32, isOutput=False)
    id_d = nc.declare_dram_parameter("ident", [128, 128], BF16, isOutput=False)
    sel_d = nc.declare_dram_parameter("sel32", [32, RPB], BF16, isOutput=False)
    # rows 0:96: expert weights zero-padded to K=32 stacked per expert
    # (expert 1 negated); rows 96:128: positional encoding routed by the
    # one-hot selector rows of xmm. Free block 512*t serves output slot t.
    wall_d = nc.declare_dram_parameter("wall", [128, TGT * D], BF16,
                                       isOutput=False)
    out_d = nc.declare_dram_parameter("out", [rows, R * TGT, D], F32,
                                      isOutput=True)

    with tile.TileContext(nc) as tc, ExitStack() as ctx:
        const = ctx.enter_context(tc.tile_pool(name="const", bufs=1))
        xin = ctx.enter_context(tc.tile_pool(name="xin", bufs=4))
        work = ctx.enter_context(tc.tile_pool(name="work", bufs=3))
        outp = ctx.enter_context(tc.tile_pool(name="outp", bufs=3))
        ps_h = ctx.enter_context(
            tc.tile_pool(name="ps_h", bufs=1, space=bass.MemorySpace.PSUM))
        ps_l = ctx.enter_context(
            tc.tile_pool(name="ps_l", bufs=1, space=bass.MemorySpace.PSUM))
        ps_m = ctx.enter_context(
            tc.tile_pool(name="ps_m", bufs=1, space=bass.MemorySpace.PSUM))
        ps_e = ctx.enter_context(
            tc.tile_pool(name="ps_e", bufs=5, space=bass.MemorySpace.PSUM))

        w1t = const.tile([96, 64], BF16)
        nc.sync.dma_start(w1t[:], w1t_d[:, :])
        b1 = const.tile([64, 1], F32)
        nc.sync.dma_start(b1[:], b1_d[:, :])
        w2d = const.tile([64, 2], BF16)
        nc.sync.dma_start(w2d[:], w2d_d[:, :])
        b2d = const.tile([RPB, 2], F32)
        nc.sync.dma_start(b2d[:], b2d_d[:, :])
        ident = const.tile([128, 128], BF16)
        nc.sync.dma_start(ident[:], id_d[:, :])
        wall = const.tile([128, TGT * D], BF16)
        nc.sync.dma_start(wall[:], wall_d[:, :])

        ov = out_d[:, :, :].rearrange("n (r t) d -> (n r) (t d)", t=TGT)

        for b in range(blocks):
            # raw x replicated on partition blocks 0:32, 32:64, 64:96
            xr = xin.tile([96, RPB], BF16, tag="xr")
            for e in range(3):
                nc.scalar.dma_start(xr[32 * e:32 * (e + 1), :],
                                    xt_d[:, RPB * b:RPB * (b + 1)])

            # classifier: h = relu(w1 @ x_region + b1)  [64, 128]
            hp = ps_h.tile([64, RPB], F32, tag="hp")
            nc.tensor.matmul(hp[:], w1t[:], xr[:], start=True, stop=True)
            ha = work.tile([64, RPB], BF16, tag="ha")
            nc.scalar.activation(ha[:], hp[:], ACTF.Relu, bias=b1[:])

            # logit differences (l0-l1, l0-l2), regions in partitions
            lp = ps_l.tile([RPB, 2], F32, tag="lp")
            nc.tensor.matmul(lp[:], ha[:], w2d[:], start=True, stop=True)

            # D01, D02 (bias applied), D12 = D02 - D01
            g = work.tile([RPB, 3], F32, tag="g")
            nc.vector.tensor_tensor(g[:, 0:2], lp[:], b2d[:], ALU.add)
            nc.vector.tensor_tensor(g[:, 2:3], g[:, 1:2], g[:, 0:1],
                                    ALU.subtract)
            cc = work.tile([RPB, 3], F32, tag="cc")
            nc.vector.tensor_scalar(cc[:], g[:], 0.0, None, ALU.is_ge)
            # one-hot masks (first-max): m0 = c0*c1, m1 = (1-c0)*c2 stored
            # negated as (c0-1)*c2, m2 = (1-c1)*(1-c2)
            ma3 = work.tile([RPB, 3], BF16, tag="ma3")
            ccm = work.tile([RPB, 2], F32, tag="ccm")
            nc.vector.tensor_tensor(ma3[:, 0:1], cc[:, 0:1], cc[:, 1:2],
                                    ALU.mult)
            nc.vector.scalar_tensor_tensor(ma3[:, 1:2], cc[:, 0:1], 1.0,
                                           cc[:, 2:3], ALU.subtract, ALU.mult)
            nc.vector.tensor_scalar(ccm[:], cc[:, 1:3], 1.0, None, ALU.subtract)
            nc.vector.tensor_tensor(ma3[:, 2:3], ccm[:, 0:1], ccm[:, 1:2],
                                    ALU.mult)

            # replicate masks and transpose+broadcast: mb[32e+j, r] = ma3[r, e]
            ma3r = work.tile([RPB, 96], BF16, tag="ma3r")
            nc.vector.tensor_copy(
                ma3r[:, :].rearrange("p (e j) -> p e j", e=3),
                ma3[:, :].broadcast_to([RPB, 3, 32]))
            mb = ps_m.tile([96, RPB], F32, tag="mb")
            nc.tensor.matmul(mb[:], ma3r[:], ident[:], start=True, stop=True)

            # masked x on rows 0:96, one-hot selector rows on 96:128
            xmm = work.tile([128, RPB], BF16, tag="xmm")
            nc.vector.tensor_tensor(xmm[0:96, :], xr[:], mb[:], ALU.mult)
            nc.scalar.dma_start(xmm[96:128, :], sel_d[:, :])

            ot = outp.tile([RPB, TGT * D], F32, tag="ot")
            for t in range(TGT):
                ps = ps_e.tile([RPB, D], F32, tag="ps")
                sl = slice(D * t, D * (t + 1))
                nc.tensor.matmul(ps[:], xmm[:], wall[:, sl],
                                 start=True, stop=True)
                nc.scalar.copy(ot[:, sl], ps[:])

            nc.sync.dma_start(ov[RPB * b:RPB * (b + 1), :], ot[:])

    nc.compile()
    return nc


def _build_wall(w_emb0, w_emb1, w_emb2):
    pe = _sinusoidal_pe(R * TGT, D)       # [128, 512] fp32
    wall = np.zeros((128, TGT * D), np.float32)
    for t in range(TGT):
        sl = slice(D * t, D * (t + 1))
        wall[8 * t:8 * t + 8, sl] = w_emb0.T
        p1 = E1_PATCH[t]
        wall[32 + 16 * p1:48 + 16 * p1, sl] = -w_emb1.T   # m1 stored negated
        wall[64:96, sl] = w_emb2.T
        # selector rows: wall[96+k, 512t+d] = pe[4k+t, d]
        wall[96:128, sl] = pe[t::TGT, :]
    return wall.astype(NPBF)


def _host_inputs(w1, b1, w2, b2, w_emb0, w_emb1, w_emb2):
    w2d = np.stack([w2[0] - w2[1], w2[0] - w2[2]], axis=1)   # [64, 2]
    b2d = np.tile(np.array([b2[0] - b2[1], b2[0] - b2[2]], np.float32),
                  (RPB, 1))
    w1t96 = np.zeros((96, 64), np.float32)
    w1t96[0:32] = w1.T
    sel = np.zeros((32, RPB), np.float32)
    sel[np.arange(RPB) % 32, np.arange(RPB)] = 1.0
    return {
        "w1t": w1t96.astype(NPBF),
        "b1c": np.ascontiguousarray(b1.reshape(64, 1)),
        "w2d": np.ascontiguousarray(w2d).astype(NPBF),
        "b2d": np.ascontiguousarray(b2d),
        "ident": np.eye(128, dtype=np.float32).astype(NPBF),
        "sel32": sel.astype(NPBF),
        "wall": _build_wall(w_emb0, w_emb1, w_emb2),
    }


_NC_CACHE = {}


def kernel(x, w1, b1, w2, b2, w_emb0, w_emb1, w_emb2, **run_kw):
    x = np.ascontiguousarray(np.asarray(x, np.float32)).reshape(N, L)
    shared = _host_inputs(
        np.asarray(w1, np.float32), np.asarray(b1, np.float32),
        np.asarray(w2, np.float32), np.asarray(b2, np.float32),
        np.asarray(w_emb0, np.float32), np.asarray(w_emb1, np.float32),
        np.asarray(w_emb2, np.float32))

    if "nc" not in _NC_CACHE:
        _NC_CACHE["nc"] = build()
    nc = _NC_CACHE["nc"]

    in_maps = []
    for i in range(NCORES):
        m = dict(shared)
        shard = x[i * ROWS:(i + 1) * ROWS]
        m["xt"] = np.ascontiguousarray(
            shard.reshape(ROWS * R, 32).T).astype(NPBF)
        in_maps.append(m)

    res = run_bass_kernel_spmd(nc, in_maps, list(range(NCORES)), **run_kw)
    full = np.concatenate([res.results[i]["out"] for i in range(NCORES)],
                          axis=0)
    kernel.last_result = res
    return full, C


if __name__ == "__main__":
    print("smoke build only")
    build(rows=8)
    print("ok")


# revision 25
# speedup vs baseline: 4.3129x; 1.0866x over previous
"""AdaptivePatchEmbedding (MoE routing) Trainium2 Bass kernel.

Full inputs in, full output out. Shards the flattened B*C=1344 row axis
across 8 NeuronCores (168 rows each); weights are replicated. The host
pre-transposes each core's x shard to xt [32 values, rows*32 regions] in
bf16 (fp32 matmuls stream at half rate and the PE clock is capped at
1.2 GHz here, so columns are precious; PSUM still accumulates fp32).
Per core the kernel processes 42 blocks of 128 regions:
  - xm [128, 128]: rows 0:96 = x replicated per expert (DMA x3), rows
    96:128 = a one-hot region-selector constant
  - classifier h = relu(w1 @ x + b1) via zero-padded w1 (full-tile
    operands only: matmuls with partition-base > 0 operands crash the
    runtime); logit differences D01/D02 via one N=2 matmul
  - first-max argmax -> one-hot masks ma3 [128reg, 3] on DVE (m1 stored
    negated, expert-1 weights negated to compensate), replicated to
    ma3r [128, 96] by a 0-stride DVE copy, transposed+broadcast to
    mb [96, 128] by one matmul against the identity
  - xmm: masked x (rows 0:96) + selector rows untouched; ONE K=128
    matmul per output slot then computes all three experts' masked
    embeddings AND adds the positional encoding (stored in the selector
    rows of the weight matrix) in a single pass: 4x512 PE columns/block
  - PSUM evicted by plain scalar-engine copies; contiguous 1 MB DMA out
"""

import math
import sys
from contextlib import ExitStack

import numpy as np

for _p in ("/opt/trn_rl_repo",):
    if _p not in sys.path:
        sys.path.insert(0, _p)

import ml_dtypes

import concourse.bass as bass
import concourse.tile as tile
from concourse import bacc, mybir
from concourse.bass_utils import run_bass_kernel_spmd

F32 = mybir.dt.float32
BF16 = mybir.dt.bfloat16
NPBF = ml_dtypes.bfloat16
ALU = mybir.AluOpType
ACTF = mybir.ActivationFunctionType

B, C, L = 64, 21, 1024
N = B * C              # 1344 rows
NCORES = 8
ROWS = N // NCORES     # 168 rows per core
R = 32                 # regions per row (L / 32)
TGT = 4                # output slots per region
D = 512                # d_model
RPB = 128              # regions per block
E1_PATCH = [0, 0, 0, 1]   # repeat_interleave(3)[:4] for n_p=2


def _sinusoidal_pe(T, DM):
    pe = np.zeros((T, DM), np.float32)
    pos = np.arange(T, dtype=np.float32)[:, None]
    div = np.exp(np.arange(0, DM, 2, dtype=np.float32) * -(math.log(10000.0) / DM))
    pe[:, 0::2] = np.sin(pos * div)
    pe[:, 1::2] = np.cos(pos * div)
    return pe


def build(rows=ROWS):
    blocks = rows * R // RPB
    nregions = rows * R
    nc = bacc.Bacc(None, target_bir_lowering=False)

    xt_d = nc.declare_dram_parameter("xt", [32, nregions], BF16, isOutput=False)
    w1t_d = nc.declare_dram_parameter("w1t", [96, 64], BF16, isOutput=False)
    b1_d = nc.declare_dram_parameter("b1c", [64, 1], F32, isOutput=False)
    w2d_d = nc.declare_dram_parameter("w2d", [64, 2], BF16, isOutput=False)
    b2d_d = nc.declare_dram_parameter("b2d", [RPB, 2], F32, isOutput=False)
    id_d = nc.declare_dram_parameter("ident", [128, 128], BF16, isOutput=False)
    sel_d = nc.declare_dram_parameter("sel32", [32, RPB], BF16, isOutput=False)
    # rows 0:96: expert weights zero-padded to K=32 stacked per expert
    # (expert 1 negated); rows 96:128: positional encoding routed by the
    # one-hot selector rows of xmm. Free block 512*t serves output slot t.
    wall_d = nc.declare_dram_parameter("wall", [128, TGT * D], BF16,
                                       isOutput=False)
    out_d = nc.declare_dram_parameter("out", [rows, R * TGT, D], F32,
                                      isOutput=True)

    with tile.TileContext(nc) as tc, ExitStack() as ctx:
        const = ctx.enter_context(tc.tile_pool(name="const", bufs=1))
        xin = ctx.enter_context(tc.tile_pool(name="xin", bufs=4))
        work = ctx.enter_context(tc.tile_pool(name="work", bufs=3))
        outp = ctx.enter_context(tc.tile_pool(name="outp", bufs=3))
        ps_h = ctx.enter_context(
            tc.tile_pool(name="ps_h", bufs=2, space=bass.MemorySpace.PSUM))
        ps_l = ctx.enter_context(
            tc.tile_pool(name="ps_l", bufs=2, space=bass.MemorySpace.PSUM))
        ps_m = ctx.enter_context(
            tc.tile_pool(name="ps_m", bufs=2, space=bass.MemorySpace.PSUM))
        ps_e = ctx.enter_context(
            tc.tile_pool(name="ps_e", bufs=2, space=bass.MemorySpace.PSUM))

        w1t = const.tile([96, 64], BF16)
        nc.sync.dma_start(w1t[:], w1t_d[:, :])
        b1 = const.tile([64, 1], F32)
        nc.sync.dma_start(b1[:], b1_d[:, :])
        w2d = const.tile([64, 2], BF16)
        nc.sync.dma_start(w2d[:], w2d_d[:, :])
        b2d = const.tile([RPB, 2], F32)
        nc.sync.dma_start(b2d[:], b2d_d[:, :])
        ident = const.tile([128, 128], BF16)
        nc.sync.dma_start(ident[:], id_d[:, :])
        wall = const.tile([128, TGT * D], BF16)
        nc.sync.dma_start(wall[:], wall_d[:, :])

        ov = out_d[:, :, :].rearrange("n (r t) d -> (n r) (t d)", t=TGT)

        GRP = 4
        for b in range(blocks):
            # raw x replicated on partition blocks 0:32, 32:64, 64:96;
            # loaded GRP blocks at a time to amortize DMA dispatch
            if b % GRP == 0:
                gw = RPB * min(GRP, blocks - b)
                xrg = xin.tile([96, RPB * GRP], BF16, tag="xrg")
                for e in range(3):
                    nc.sync.dma_start(xrg[32 * e:32 * (e + 1), 0:gw],
                                      xt_d[:, RPB * b:RPB * b + gw])
            xr = xrg[:, RPB * (b % GRP):RPB * (b % GRP + 1)]

            # classifier: h = relu(w1 @ x_region + b1)  [64, 128]
            hp = ps_h.tile([64, RPB], F32, tag="hp")
            nc.tensor.matmul(hp[:], w1t[:], xr, start=True, stop=True)
            ha = work.tile([64, RPB], BF16, tag="ha")
            nc.scalar.activation(ha[:], hp[:], ACTF.Relu, bias=b1[:])

            # logit differences (l0-l1, l0-l2), regions in partitions
            lp = ps_l.tile([RPB, 2], F32, tag="lp")
            nc.tensor.matmul(lp[:], ha[:], w2d[:], start=True, stop=True)

            # D01, D02 (bias applied), D12 = D02 - D01
            g = work.tile([RPB, 3], F32, tag="g")
            nc.vector.tensor_tensor(g[:, 0:2], lp[:], b2d[:], ALU.add)
            nc.vector.tensor_tensor(g[:, 2:3], g[:, 1:2], g[:, 0:1],
                                    ALU.subtract)
            cc = work.tile([RPB, 3], F32, tag="cc")
            nc.vector.tensor_scalar(cc[:], g[:], 0.0, None, ALU.is_ge)
            # one-hot masks (first-max): m0 = c0*c1, m1 = (1-c0)*c2 stored
            # negated as (c0-1)*c2, m2 = (1-c1)*(1-c2)
            ma3 = work.tile([RPB, 3], BF16, tag="ma3")
            ccm = work.tile([RPB, 2], F32, tag="ccm")
            nc.vector.tensor_tensor(ma3[:, 0:1], cc[:, 0:1], cc[:, 1:2],
                                    ALU.mult)
            nc.vector.scalar_tensor_tensor(ma3[:, 1:2], cc[:, 0:1], 1.0,
                                           cc[:, 2:3], ALU.subtract, ALU.mult)
            nc.vector.tensor_scalar(ccm[:], cc[:, 1:3], 1.0, None, ALU.subtract)
            nc.vector.tensor_tensor(ma3[:, 2:3], ccm[:, 0:1], ccm[:, 1:2],
                                    ALU.mult)

            # replicate masks and transpose+broadcast: mb[32e+j, r] = ma3[r, e]
            ma3r = work.tile([RPB, 96], BF16, tag="ma3r")
            nc.vector.tensor_copy(
                ma3r[:, :].rearrange("p (e j) -> p e j", e=3),
                ma3[:, :].broadcast_to([RPB, 3, 32]))
            mb = ps_m.tile([96, RPB], F32, tag="mb")
            nc.tensor.matmul(mb[:], ma3r[:], ident[:], start=True, stop=True)

            # masked x on rows 0:96, one-hot selector rows on 96:128
            xmm = work.tile([128, RPB], BF16, tag="xmm")
            nc.vector.tensor_tensor(xmm[0:96, :], xr, mb[:], ALU.mult)
            nc.sync.dma_start(xmm[96:128, :], sel_d[:, :])

            ot = outp.tile([RPB, TGT * D], F32, tag="ot")
            for t in range(TGT):
                ps = ps_e.tile([RPB, D], F32, tag="ps")
                sl = slice(D * t, D * (t + 1))
                nc.tensor.matmul(ps[:], xmm[:], wall[:, sl],
                                 start=True, stop=True)
                if t == 0:
                    nc.vector.tensor_copy(ot[:, sl], ps[:])
                else:
                    nc.scalar.copy(ot[:, sl], ps[:])

            nc.sync.dma_start(ov[RPB * b:RPB * (b + 1), :], ot[:])

    nc.compile()
    return nc


def _build_wall(w_emb0, w_emb1, w_emb2):
    pe = _sinusoidal_pe(R * TGT, D)       # [128, 512] fp32
    wall = np.zeros((128, TGT * D), np.float32)
    for t in range(TGT):
        sl = slice(D * t, D * (t + 1))
        wall[8 * t:8 * t + 8, sl] = w_emb0.T
        p1 = E1_PATCH[t]
        wall[32 + 16 * p1:48 + 16 * p1, sl] = -w_emb1.T   # m1 stored negated
        wall[64:96, sl] = w_emb2.T
        # selector rows: wall[96+k, 512t+d] = pe[4k+t, d]
        wall[96:128, sl] = pe[t::TGT, :]
    return wall.astype(NPBF)


def _host_inputs(w1, b1, w2, b2, w_emb0, w_emb1, w_emb2):
    w2d = np.stack([w2[0] - w2[1], w2[0] - w2[2]], axis=1)   # [64, 2]
    b2d = np.tile(np.array([b2[0] - b2[1], b2[0] - b2[2]], np.float32),
                  (RPB, 1))
    w1t96 = np.zeros((96, 64), np.float32)
    w1t96[0:32] = w1.T
    sel = np.zeros((32, RPB), np.float32)
    sel[np.arange(RPB) % 32, np.arange(RPB)] = 1.0
    return {
        "w1t": w1t96.astype(NPBF),
        "b1c": np.ascontiguousarray(b1.reshape(64, 1)),
        "w2d": np.ascontiguousarray(w2d).astype(NPBF),
        "b2d": np.ascontiguousarray(b2d),
        "ident": np.eye(128, dtype=np.float32).astype(NPBF),
        "sel32": sel.astype(NPBF),
        "wall": _build_wall(w_emb0, w_emb1, w_emb2),
    }


_NC_CACHE = {}


def kernel(x, w1, b1, w2, b2, w_emb0, w_emb1, w_emb2, **run_kw):
    x = np.ascontiguousarray(np.asarray(x, np.float32)).reshape(N, L)
    shared = _host_inputs(
        np.asarray(w1, np.float32), np.asarray(b1, np.float32),
        np.asarray(w2, np.float32), np.asarray(b2, np.float32),
        np.asarray(w_emb0, np.float32), np.asarray(w_emb1, np.float32),
        np.asarray(w_emb2, np.float32))

    if "nc" not in _NC_CACHE:
        _NC_CACHE["nc"] = build()
    nc = _NC_CACHE["nc"]

    in_maps = []
    for i in range(NCORES):
        m = dict(shared)
        shard = x[i * ROWS:(i + 1) * ROWS]
        m["xt"] = np.ascontiguousarray(
            shard.reshape(ROWS * R, 32).T).astype(NPBF)
        in_maps.append(m)

    res = run_bass_kernel_spmd(nc, in_maps, list(range(NCORES)), **run_kw)
    full = np.concatenate([res.results[i]["out"] for i in range(NCORES)],
                          axis=0)
    kernel.last_result = res
    return full, C


if __name__ == "__main__":
    print("smoke build only")
    build(rows=8)
    print("ok")


# revision 26
# speedup vs baseline: 4.5997x; 1.0665x over previous
"""AdaptivePatchEmbedding (MoE routing) Trainium2 Bass kernel.

Full inputs in, full output out. Shards the flattened B*C=1344 row axis
across 8 NeuronCores (168 rows each); weights are replicated. The host
pre-transposes each core's x shard to xt [32 values, rows*32 regions] in
bf16 (fp32 matmuls stream at half rate and the PE clock is capped at
1.2 GHz here, so columns are precious; PSUM still accumulates fp32).
Per core the kernel processes 42 blocks of 128 regions:
  - xm [128, 128]: rows 0:96 = x replicated per expert (DMA x3), rows
    96:128 = a one-hot region-selector constant
  - classifier h = relu(w1 @ x + b1) via zero-padded w1 (full-tile
    operands only: matmuls with partition-base > 0 operands crash the
    runtime); logit differences D01/D02 via one N=2 matmul
  - first-max argmax -> one-hot masks ma3 [128reg, 3] on DVE (m1 stored
    negated, expert-1 weights negated to compensate), replicated to
    ma3r [128, 96] by a 0-stride DVE copy, transposed+broadcast to
    mb [96, 128] by one matmul against the identity
  - xmm: masked x (rows 0:96) + selector rows untouched; ONE K=128
    matmul per output slot then computes all three experts' masked
    embeddings AND adds the positional encoding (stored in the selector
    rows of the weight matrix) in a single pass: 4x512 PE columns/block
  - PSUM evicted by plain scalar-engine copies; contiguous 1 MB DMA out
"""

import math
import sys
from contextlib import ExitStack

import numpy as np

for _p in ("/opt/trn_rl_repo",):
    if _p not in sys.path:
        sys.path.insert(0, _p)

import ml_dtypes

import concourse.bass as bass
import concourse.tile as tile
from concourse import bacc, mybir
from concourse.bass_utils import run_bass_kernel_spmd

F32 = mybir.dt.float32
BF16 = mybir.dt.bfloat16
NPBF = ml_dtypes.bfloat16
ALU = mybir.AluOpType
ACTF = mybir.ActivationFunctionType

B, C, L = 64, 21, 1024
N = B * C              # 1344 rows
NCORES = 8
ROWS = N // NCORES     # 168 rows per core
R = 32                 # regions per row (L / 32)
TGT = 4                # output slots per region
D = 512                # d_model
RPB = 128              # regions per block
E1_PATCH = [0, 0, 0, 1]   # repeat_interleave(3)[:4] for n_p=2


def _sinusoidal_pe(T, DM):
    pe = np.zeros((T, DM), np.float32)
    pos = np.arange(T, dtype=np.float32)[:, None]
    div = np.exp(np.arange(0, DM, 2, dtype=np.float32) * -(math.log(10000.0) / DM))
    pe[:, 0::2] = np.sin(pos * div)
    pe[:, 1::2] = np.cos(pos * div)
    return pe


def build(rows=ROWS):
    blocks = rows * R // RPB
    nregions = rows * R
    nc = bacc.Bacc(None, target_bir_lowering=False)

    xt_d = nc.declare_dram_parameter("xt", [32, nregions], BF16, isOutput=False)
    w1t_d = nc.declare_dram_parameter("w1t", [96, 64], BF16, isOutput=False)
    b1_d = nc.declare_dram_parameter("b1c", [64, 1], F32, isOutput=False)
    w2d_d = nc.declare_dram_parameter("w2d", [96, 3], BF16, isOutput=False)
    id_d = nc.declare_dram_parameter("ident", [128, 128], BF16, isOutput=False)
    sel_d = nc.declare_dram_parameter("sel32", [32, RPB], BF16, isOutput=False)
    # rows 0:96: expert weights zero-padded to K=32 stacked per expert
    # (expert 1 negated); rows 96:128: positional encoding routed by the
    # one-hot selector rows of xmm. Free block 512*t serves output slot t.
    wall_d = nc.declare_dram_parameter("wall", [128, TGT * D], BF16,
                                       isOutput=False)
    out_d = nc.declare_dram_parameter("out", [rows, R * TGT, D], F32,
                                      isOutput=True)

    with tile.TileContext(nc) as tc, ExitStack() as ctx:
        const = ctx.enter_context(tc.tile_pool(name="const", bufs=1))
        xin = ctx.enter_context(tc.tile_pool(name="xin", bufs=4))
        work = ctx.enter_context(tc.tile_pool(name="work", bufs=4))
        outp = ctx.enter_context(tc.tile_pool(name="outp", bufs=3))
        ps_s = ctx.enter_context(
            tc.tile_pool(name="ps_s", bufs=4, space=bass.MemorySpace.PSUM))
        ps_e = ctx.enter_context(
            tc.tile_pool(name="ps_e", bufs=4, space=bass.MemorySpace.PSUM))

        w1t = const.tile([96, 64], BF16)
        nc.sync.dma_start(w1t[:], w1t_d[:, :])
        b1 = const.tile([64, 1], F32)
        nc.sync.dma_start(b1[:], b1_d[:, :])
        w2d = const.tile([96, 3], BF16)
        nc.sync.dma_start(w2d[:], w2d_d[:, :])
        ident = const.tile([128, 128], BF16)
        nc.sync.dma_start(ident[:], id_d[:, :])
        wall = const.tile([128, TGT * D], BF16)
        nc.sync.dma_start(wall[:], wall_d[:, :])

        ov = out_d[:, :, :].rearrange("n (r t) d -> (n r) (t d)", t=TGT)

        GRP = 4
        for b in range(blocks):
            # raw x replicated on partition blocks 0:32, 32:64, 64:96;
            # loaded GRP blocks at a time to amortize DMA dispatch
            if b % GRP == 0:
                gw = RPB * min(GRP, blocks - b)
                xrg = xin.tile([96, RPB * GRP], BF16, tag="xrg")
                for e in range(3):
                    nc.sync.dma_start(xrg[32 * e:32 * (e + 1), 0:gw],
                                      xt_d[:, RPB * b:RPB * b + gw])
            xr = xrg[:, RPB * (b % GRP):RPB * (b % GRP + 1)]

            # classifier: h = relu(w1 @ x_region + b1); rows 64:96 are an
            # all-ones block so the augmented w2d rows add the b2 bias
            hp = ps_s.tile([64, RPB], F32, tag="ps_s")
            nc.tensor.matmul(hp[:], w1t[:], xr, start=True, stop=True)
            ha = work.tile([96, RPB], BF16, tag="ha")
            nc.scalar.activation(ha[0:64, :], hp[:], ACTF.Relu, bias=b1[:])
            nc.gpsimd.memset(ha[64:96, :], 1.0)

            # biased logit differences (D01, D02, D12), regions in partitions
            lp = ps_s.tile([RPB, 3], F32, tag="ps_s")
            nc.tensor.matmul(lp[:], ha[:], w2d[:], start=True, stop=True)
            cc = work.tile([RPB, 3], F32, tag="cc")
            nc.vector.tensor_scalar(cc[:], lp[:], 0.0, None, ALU.is_ge)
            # one-hot masks (first-max): m0 = c0*c1, m1 = (1-c0)*c2 stored
            # negated as (c0-1)*c2, m2 = (1-c1)*(1-c2)
            ma3 = work.tile([RPB, 3], BF16, tag="ma3")
            ccm = work.tile([RPB, 2], F32, tag="ccm")
            nc.vector.tensor_tensor(ma3[:, 0:1], cc[:, 0:1], cc[:, 1:2],
                                    ALU.mult)
            nc.vector.scalar_tensor_tensor(ma3[:, 1:2], cc[:, 0:1], 1.0,
                                           cc[:, 2:3], ALU.subtract, ALU.mult)
            nc.vector.tensor_scalar(ccm[:], cc[:, 1:3], 1.0, None, ALU.subtract)
            nc.vector.tensor_tensor(ma3[:, 2:3], ccm[:, 0:1], ccm[:, 1:2],
                                    ALU.mult)

            # replicate masks and transpose+broadcast: mb[32e+j, r] = ma3[r, e]
            ma3r = work.tile([RPB, 96], BF16, tag="ma3r")
            nc.vector.tensor_copy(
                ma3r[:, :].rearrange("p (e j) -> p e j", e=3),
                ma3[:, :].broadcast_to([RPB, 3, 32]))
            mb = ps_s.tile([96, RPB], F32, tag="ps_s")
            nc.tensor.matmul(mb[:], ma3r[:], ident[:], start=True, stop=True)

            # masked x on rows 0:96, one-hot selector rows on 96:128
            xmm = work.tile([128, RPB], BF16, tag="xmm")
            nc.vector.tensor_tensor(xmm[0:96, :], xr, mb[:], ALU.mult)
            nc.sync.dma_start(xmm[96:128, :], sel_d[:, :])

            ot = outp.tile([RPB, TGT * D], F32, tag="ot")
            for t in range(TGT):
                ps = ps_e.tile([RPB, D], F32, tag="ps")
                sl = slice(D * t, D * (t + 1))
                nc.tensor.matmul(ps[:], xmm[:], wall[:, sl],
                                 start=True, stop=True)
                if t < 2:
                    nc.vector.tensor_copy(ot[:, sl], ps[:])
                else:
                    nc.scalar.copy(ot[:, sl], ps[:])

            nc.sync.dma_start(ov[RPB * b:RPB * (b + 1), :], ot[:])

    nc.compile()
    return nc


def _build_wall(w_emb0, w_emb1, w_emb2):
    pe = _sinusoidal_pe(R * TGT, D)       # [128, 512] fp32
    wall = np.zeros((128, TGT * D), np.float32)
    for t in range(TGT):
        sl = slice(D * t, D * (t + 1))
        wall[8 * t:8 * t + 8, sl] = w_emb0.T
        p1 = E1_PATCH[t]
        wall[32 + 16 * p1:48 + 16 * p1, sl] = -w_emb1.T   # m1 stored negated
        wall[64:96, sl] = w_emb2.T
        # selector rows: wall[96+k, 512t+d] = pe[4k+t, d]
        wall[96:128, sl] = pe[t::TGT, :]
    return wall.astype(NPBF)


def _host_inputs(w1, b1, w2, b2, w_emb0, w_emb1, w_emb2):
    w2d = np.zeros((96, 3), np.float32)
    w2d[0:64, 0] = w2[0] - w2[1]
    w2d[0:64, 1] = w2[0] - w2[2]
    w2d[0:64, 2] = w2[1] - w2[2]
    # the ones-rows of ha are 32 wide; spread the bias over them
    w2d[64, 0] = b2[0] - b2[1]
    w2d[64, 1] = b2[0] - b2[2]
    w2d[64, 2] = b2[1] - b2[2]
    w1t96 = np.zeros((96, 64), np.float32)
    w1t96[0:32] = w1.T
    sel = np.zeros((32, RPB), np.float32)
    sel[np.arange(RPB) % 32, np.arange(RPB)] = 1.0
    return {
        "w1t": w1t96.astype(NPBF),
        "b1c": np.ascontiguousarray(b1.reshape(64, 1)),
        "w2d": np.ascontiguousarray(w2d).astype(NPBF),
        "ident": np.eye(128, dtype=np.float32).astype(NPBF),
        "sel32": sel.astype(NPBF),
        "wall": _build_wall(w_emb0, w_emb1, w_emb2),
    }


_NC_CACHE = {}


def kernel(x, w1, b1, w2, b2, w_emb0, w_emb1, w_emb2, **run_kw):
    x = np.ascontiguousarray(np.asarray(x, np.float32)).reshape(N, L)
    shared = _host_inputs(
        np.asarray(w1, np.float32), np.asarray(b1, np.float32),
        np.asarray(w2, np.float32), np.asarray(b2, np.float32),
        np.asarray(w_emb0, np.float32), np.asarray(w_emb1, np.float32),
        np.asarray(w_emb2, np.float32))

    if "nc" not in _NC_CACHE:
        _NC_CACHE["nc"] = build()
    nc = _NC_CACHE["nc"]

    in_maps = []
    for i in range(NCORES):
        m = dict(shared)
        shard = x[i * ROWS:(i + 1) * ROWS]
        m["xt"] = np.ascontiguousarray(
            shard.reshape(ROWS * R, 32).T).astype(NPBF)
        in_maps.append(m)

    res = run_bass_kernel_spmd(nc, in_maps, list(range(NCORES)), **run_kw)
    full = np.concatenate([res.results[i]["out"] for i in range(NCORES)],
                          axis=0)
    kernel.last_result = res
    return full, C


if __name__ == "__main__":
    print("smoke build only")
    build(rows=8)
    print("ok")


# revision 27
# speedup vs baseline: 5.6583x; 1.2301x over previous
"""AdaptivePatchEmbedding (MoE routing) Trainium2 Bass kernel.

Full inputs in, full output out. Shards the flattened B*C=1344 row axis
across 8 NeuronCores (168 rows each); weights are replicated. The host
pre-transposes each core's x shard to xt [32 values, rows*32 regions] in
bf16 (fp32 matmuls stream at half rate and the PE clock is capped at
1.2 GHz here, so columns are precious; PSUM still accumulates fp32).
Per core the kernel processes 42 blocks of 128 regions:
  - xm [128, 128]: rows 0:96 = x replicated per expert (DMA x3), rows
    96:128 = a one-hot region-selector constant
  - classifier h = relu(w1 @ x + b1) via zero-padded w1 (full-tile
    operands only: matmuls with partition-base > 0 operands crash the
    runtime); logit differences D01/D02 via one N=2 matmul
  - first-max argmax -> one-hot masks ma3 [128reg, 3] on DVE (m1 stored
    negated, expert-1 weights negated to compensate), replicated to
    ma3r [128, 96] by a 0-stride DVE copy, transposed+broadcast to
    mb [96, 128] by one matmul against the identity
  - xmm: masked x (rows 0:96) + selector rows untouched; ONE K=128
    matmul per output slot then computes all three experts' masked
    embeddings AND adds the positional encoding (stored in the selector
    rows of the weight matrix) in a single pass: 4x512 PE columns/block
  - PSUM evicted by plain scalar-engine copies; contiguous 1 MB DMA out
"""

import math
import sys
from contextlib import ExitStack

import numpy as np

for _p in ("/opt/trn_rl_repo",):
    if _p not in sys.path:
        sys.path.insert(0, _p)

import ml_dtypes

import concourse.bass as bass
import concourse.tile as tile
from concourse import bacc, mybir
from concourse.bass_utils import run_bass_kernel_spmd

F32 = mybir.dt.float32
BF16 = mybir.dt.bfloat16
NPBF = ml_dtypes.bfloat16
ALU = mybir.AluOpType
ACTF = mybir.ActivationFunctionType

B, C, L = 64, 21, 1024
N = B * C              # 1344 rows
NCORES = 8
ROWS = N // NCORES     # 168 rows per core
R = 32                 # regions per row (L / 32)
TGT = 4                # output slots per region
D = 512                # d_model
RPB = 128              # regions per block
E1_PATCH = [0, 0, 0, 1]   # repeat_interleave(3)[:4] for n_p=2


def _sinusoidal_pe(T, DM):
    pe = np.zeros((T, DM), np.float32)
    pos = np.arange(T, dtype=np.float32)[:, None]
    div = np.exp(np.arange(0, DM, 2, dtype=np.float32) * -(math.log(10000.0) / DM))
    pe[:, 0::2] = np.sin(pos * div)
    pe[:, 1::2] = np.cos(pos * div)
    return pe


def build(rows=ROWS):
    blocks = rows * R // RPB
    nregions = rows * R
    nc = bacc.Bacc(None, target_bir_lowering=False)

    xt_d = nc.declare_dram_parameter("xt", [32, nregions], BF16, isOutput=False)
    w1t_d = nc.declare_dram_parameter("w1t", [96, 64], BF16, isOutput=False)
    b1_d = nc.declare_dram_parameter("b1c", [64, 1], F32, isOutput=False)
    w2d_d = nc.declare_dram_parameter("w2d", [96, 3], BF16, isOutput=False)
    id_d = nc.declare_dram_parameter("ident", [128, 128], BF16, isOutput=False)
    sel_d = nc.declare_dram_parameter("sel32", [32, RPB], BF16, isOutput=False)
    # rows 0:96: expert weights zero-padded to K=32 stacked per expert
    # (expert 1 negated); rows 96:128: positional encoding routed by the
    # one-hot selector rows of xmm. Free block 512*t serves output slot t.
    wall_d = nc.declare_dram_parameter("wall", [128, TGT * D], BF16,
                                       isOutput=False)
    out_d = nc.declare_dram_parameter("out", [rows, R * TGT, D], F32,
                                      isOutput=True)

    with tile.TileContext(nc) as tc, ExitStack() as ctx:
        const = ctx.enter_context(tc.tile_pool(name="const", bufs=1))
        xin = ctx.enter_context(tc.tile_pool(name="xin", bufs=4))
        work = ctx.enter_context(tc.tile_pool(name="work", bufs=4))
        outp = ctx.enter_context(tc.tile_pool(name="outp", bufs=4))
        ps_s = ctx.enter_context(
            tc.tile_pool(name="ps_s", bufs=4, space=bass.MemorySpace.PSUM))
        ps_e = ctx.enter_context(
            tc.tile_pool(name="ps_e", bufs=4, space=bass.MemorySpace.PSUM))

        w1t = const.tile([96, 64], BF16)
        nc.sync.dma_start(w1t[:], w1t_d[:, :])
        b1 = const.tile([64, 1], F32)
        nc.sync.dma_start(b1[:], b1_d[:, :])
        w2d = const.tile([96, 3], BF16)
        nc.sync.dma_start(w2d[:], w2d_d[:, :])
        ident = const.tile([128, 128], BF16)
        nc.sync.dma_start(ident[:], id_d[:, :])
        wall = const.tile([128, TGT * D], BF16)
        nc.sync.dma_start(wall[:], wall_d[:, :])

        ov = out_d[:, :, :].rearrange("n (r t) d -> (n r) (t d)", t=TGT)

        GRP = 6
        for b in range(blocks):
            # raw x replicated on partition blocks 0:32, 32:64, 64:96;
            # loaded GRP blocks at a time to amortize DMA dispatch
            if b % GRP == 0:
                gw = RPB * min(GRP, blocks - b)
                xrg = xin.tile([96, RPB * GRP], BF16, tag="xrg")
                for e in range(3):
                    nc.scalar.dma_start(xrg[32 * e:32 * (e + 1), 0:gw],
                                        xt_d[:, RPB * b:RPB * b + gw])
            xr = xrg[:, RPB * (b % GRP):RPB * (b % GRP + 1)]

            # classifier: h = relu(w1 @ x_region + b1); rows 64:96 are an
            # all-ones block so the augmented w2d rows add the b2 bias
            hp = ps_s.tile([64, RPB], F32, tag="ps_s")
            nc.tensor.matmul(hp[:], w1t[:], xr, start=True, stop=True)
            ha = work.tile([96, RPB], BF16, tag="ha")
            nc.scalar.activation(ha[0:64, :], hp[:], ACTF.Relu, bias=b1[:])
            nc.gpsimd.memset(ha[64:96, :], 1.0)

            # biased logit differences (D01, D02, D12), regions in partitions
            lp = ps_s.tile([RPB, 3], F32, tag="ps_s")
            nc.tensor.matmul(lp[:], ha[:], w2d[:], start=True, stop=True)
            cc = work.tile([RPB, 3], F32, tag="cc")
            nc.vector.tensor_scalar(cc[:], lp[:], 0.0, None, ALU.is_ge)
            # one-hot masks (first-max): m0 = c0*c1, m1 = (1-c0)*c2 stored
            # negated as (c0-1)*c2, m2 = (1-c1)*(1-c2)
            ma3 = work.tile([RPB, 3], BF16, tag="ma3")
            ccm = work.tile([RPB, 2], F32, tag="ccm")
            nc.vector.tensor_tensor(ma3[:, 0:1], cc[:, 0:1], cc[:, 1:2],
                                    ALU.mult)
            nc.vector.scalar_tensor_tensor(ma3[:, 1:2], cc[:, 0:1], 1.0,
                                           cc[:, 2:3], ALU.subtract, ALU.mult)
            nc.vector.tensor_scalar(ccm[:], cc[:, 1:3], 1.0, None, ALU.subtract)
            nc.vector.tensor_tensor(ma3[:, 2:3], ccm[:, 0:1], ccm[:, 1:2],
                                    ALU.mult)

            # replicate masks and transpose+broadcast: mb[32e+j, r] = ma3[r, e]
            ma3r = work.tile([RPB, 96], BF16, tag="ma3r")
            nc.vector.tensor_copy(
                ma3r[:, :].rearrange("p (e j) -> p e j", e=3),
                ma3[:, :].broadcast_to([RPB, 3, 32]))
            mb = ps_s.tile([96, RPB], F32, tag="ps_s")
            nc.tensor.matmul(mb[:], ma3r[:], ident[:], start=True, stop=True)

            # masked x on rows 0:96, one-hot selector rows on 96:128
            xmm = work.tile([128, RPB], BF16, tag="xmm")
            nc.vector.tensor_tensor(xmm[0:96, :], xr, mb[:], ALU.mult)
            nc.scalar.dma_start(xmm[96:128, :], sel_d[:, :])

            ot = outp.tile([RPB, TGT * D], F32, tag="ot")
            for t in range(TGT):
                ps = ps_e.tile([RPB, D], F32, tag="ps")
                sl = slice(D * t, D * (t + 1))
                nc.tensor.matmul(ps[:], xmm[:], wall[:, sl],
                                 start=True, stop=True)
                if t < 2:
                    nc.vector.tensor_copy(ot[:, sl], ps[:])
                else:
                    nc.scalar.copy(ot[:, sl], ps[:])

            nc.sync.dma_start(ov[RPB * b:RPB * (b + 1), :], ot[:])

    nc.compile()
    return nc


def _build_wall(w_emb0, w_emb1, w_emb2):
    pe = _sinusoidal_pe(R * TGT, D)       # [128, 512] fp32
    wall = np.zeros((128, TGT * D), np.float32)
    for t in range(TGT):
        sl = slice(D * t, D * (t + 1))
        wall[8 * t:8 * t + 8, sl] = w_emb0.T
        p1 = E1_PATCH[t]
        wall[32 + 16 * p1:48 + 16 * p1, sl] = -w_emb1.T   # m1 stored negated
        wall[64:96, sl] = w_emb2.T
        # selector rows: wall[96+k, 512t+d] = pe[4k+t, d]
        wall[96:128, sl] = pe[t::TGT, :]
    return wall.astype(NPBF)


def _host_inputs(w1, b1, w2, b2, w_emb0, w_emb1, w_emb2):
    w2d = np.zeros((96, 3), np.float32)
    w2d[0:64, 0] = w2[0] - w2[1]
    w2d[0:64, 1] = w2[0] - w2[2]
    w2d[0:64, 2] = w2[1] - w2[2]
    # the ones-rows of ha are 32 wide; spread the bias over them
    w2d[64, 0] = b2[0] - b2[1]
    w2d[64, 1] = b2[0] - b2[2]
    w2d[64, 2] = b2[1] - b2[2]
    w1t96 = np.zeros((96, 64), np.float32)
    w1t96[0:32] = w1.T
    sel = np.zeros((32, RPB), np.float32)
    sel[np.arange(RPB) % 32, np.arange(RPB)] = 1.0
    return {
        "w1t": w1t96.astype(NPBF),
        "b1c": np.ascontiguousarray(b1.reshape(64, 1)),
        "w2d": np.ascontiguousarray(w2d).astype(NPBF),
        "ident": np.eye(128, dtype=np.float32).astype(NPBF),
        "sel32": sel.astype(NPBF),
        "wall": _build_wall(w_emb0, w_emb1, w_emb2),
    }


_NC_CACHE = {}


def kernel(x, w1, b1, w2, b2, w_emb0, w_emb1, w_emb2, **run_kw):
    x = np.ascontiguousarray(np.asarray(x, np.float32)).reshape(N, L)
    shared = _host_inputs(
        np.asarray(w1, np.float32), np.asarray(b1, np.float32),
        np.asarray(w2, np.float32), np.asarray(b2, np.float32),
        np.asarray(w_emb0, np.float32), np.asarray(w_emb1, np.float32),
        np.asarray(w_emb2, np.float32))

    if "nc" not in _NC_CACHE:
        _NC_CACHE["nc"] = build()
    nc = _NC_CACHE["nc"]

    in_maps = []
    for i in range(NCORES):
        m = dict(shared)
        shard = x[i * ROWS:(i + 1) * ROWS]
        m["xt"] = np.ascontiguousarray(
            shard.reshape(ROWS * R, 32).T).astype(NPBF)
        in_maps.append(m)

    res = run_bass_kernel_spmd(nc, in_maps, list(range(NCORES)), **run_kw)
    full = np.concatenate([res.results[i]["out"] for i in range(NCORES)],
                          axis=0)
    kernel.last_result = res
    return full, C


if __name__ == "__main__":
    print("smoke build only")
    build(rows=8)
    print("ok")


# revision 28
# speedup vs baseline: 6.1869x; 1.0934x over previous
"""AdaptivePatchEmbedding (MoE routing) Trainium2 Bass kernel.

Full inputs in, full output out. Shards the flattened B*C=1344 row axis
across 8 NeuronCores (168 rows each); weights are replicated. The host
pre-transposes each core's x shard to xt [32 values, rows*32 regions] in
bf16 (fp32 matmuls stream at half rate and the PE clock is capped at
1.2 GHz here, so columns are precious; PSUM still accumulates fp32).
Per core the kernel processes 42 blocks of 128 regions:
  - xm [128, 128]: rows 0:96 = x replicated per expert (DMA x3), rows
    96:128 = a one-hot region-selector constant
  - classifier h = relu(w1 @ x + b1) via zero-padded w1 (full-tile
    operands only: matmuls with partition-base > 0 operands crash the
    runtime); logit differences D01/D02 via one N=2 matmul
  - first-max argmax -> one-hot masks ma3 [128reg, 3] on DVE (m1 stored
    negated, expert-1 weights negated to compensate), replicated to
    ma3r [128, 96] by a 0-stride DVE copy, transposed+broadcast to
    mb [96, 128] by one matmul against the identity
  - xmm: masked x (rows 0:96) + selector rows untouched; ONE K=128
    matmul per output slot then computes all three experts' masked
    embeddings AND adds the positional encoding (stored in the selector
    rows of the weight matrix) in a single pass: 4x512 PE columns/block
  - PSUM evicted by plain scalar-engine copies; contiguous 1 MB DMA out
"""

import math
import sys
from contextlib import ExitStack

import numpy as np

for _p in ("/opt/trn_rl_repo",):
    if _p not in sys.path:
        sys.path.insert(0, _p)

import ml_dtypes

import concourse.bass as bass
import concourse.tile as tile
from concourse import bacc, mybir
from concourse.bass_utils import run_bass_kernel_spmd

F32 = mybir.dt.float32
BF16 = mybir.dt.bfloat16
NPBF = ml_dtypes.bfloat16
ALU = mybir.AluOpType
ACTF = mybir.ActivationFunctionType

B, C, L = 64, 21, 1024
N = B * C              # 1344 rows
NCORES = 8
ROWS = N // NCORES     # 168 rows per core
R = 32                 # regions per row (L / 32)
TGT = 4                # output slots per region
D = 512                # d_model
RPB = 128              # regions per block
E1_PATCH = [0, 0, 0, 1]   # repeat_interleave(3)[:4] for n_p=2


def _sinusoidal_pe(T, DM):
    pe = np.zeros((T, DM), np.float32)
    pos = np.arange(T, dtype=np.float32)[:, None]
    div = np.exp(np.arange(0, DM, 2, dtype=np.float32) * -(math.log(10000.0) / DM))
    pe[:, 0::2] = np.sin(pos * div)
    pe[:, 1::2] = np.cos(pos * div)
    return pe


def build(rows=ROWS):
    blocks = rows * R // RPB
    nregions = rows * R
    nc = bacc.Bacc(None, target_bir_lowering=False)

    xt_d = nc.declare_dram_parameter("xt", [32, nregions], BF16, isOutput=False)
    w1t_d = nc.declare_dram_parameter("w1t", [96, 64], BF16, isOutput=False)
    b1_d = nc.declare_dram_parameter("b1c", [64, 1], F32, isOutput=False)
    w2d_d = nc.declare_dram_parameter("w2d", [96, 3], BF16, isOutput=False)
    id_d = nc.declare_dram_parameter("ident", [128, 128], BF16, isOutput=False)
    sel_d = nc.declare_dram_parameter("sel32", [32, RPB], BF16, isOutput=False)
    # rows 0:96: expert weights zero-padded to K=32 stacked per expert
    # (expert 1 negated); rows 96:128: positional encoding routed by the
    # one-hot selector rows of xmm. Free block 512*t serves output slot t.
    wall_d = nc.declare_dram_parameter("wall", [128, TGT * D], BF16,
                                       isOutput=False)
    out_d = nc.declare_dram_parameter("out", [rows, R * TGT, D], F32,
                                      isOutput=True)

    with tile.TileContext(nc) as tc, ExitStack() as ctx:
        const = ctx.enter_context(tc.tile_pool(name="const", bufs=1))
        xin = ctx.enter_context(tc.tile_pool(name="xin", bufs=4))
        work = ctx.enter_context(tc.tile_pool(name="work", bufs=4))
        outp = ctx.enter_context(tc.tile_pool(name="outp", bufs=4))
        ps_s = ctx.enter_context(
            tc.tile_pool(name="ps_s", bufs=4, space=bass.MemorySpace.PSUM))
        ps_e = ctx.enter_context(
            tc.tile_pool(name="ps_e", bufs=4, space=bass.MemorySpace.PSUM))

        w1t = const.tile([96, 64], BF16)
        nc.sync.dma_start(w1t[:], w1t_d[:, :])
        b1 = const.tile([64, 1], F32)
        nc.sync.dma_start(b1[:], b1_d[:, :])
        w2d = const.tile([96, 3], BF16)
        nc.sync.dma_start(w2d[:], w2d_d[:, :])
        ident = const.tile([128, 128], BF16)
        nc.sync.dma_start(ident[:], id_d[:, :])
        wall = const.tile([128, TGT * D], BF16)
        nc.sync.dma_start(wall[:], wall_d[:, :])

        ov = out_d[:, :, :].rearrange("n (r t) d -> (n r) (t d)", t=TGT)

        GRP = 6
        state = {}

        def stage_a(b):
            # classifier + routing masks for block b -> (ma3r, xr)
            if b % GRP == 0:
                gw = RPB * min(GRP, blocks - b)
                xrg = xin.tile([96, RPB * GRP], BF16, tag="xrg")
                for e in range(3):
                    nc.scalar.dma_start(xrg[32 * e:32 * (e + 1), 0:gw],
                                        xt_d[:, RPB * b:RPB * b + gw])
                state["xrg"] = xrg
            xr = state["xrg"][:, RPB * (b % GRP):RPB * (b % GRP + 1)]

            # h = relu(w1 @ x + b1); rows 64:96 all-ones so the augmented
            # w2d rows add the b2 bias
            hp = ps_s.tile([64, RPB], F32, tag="ps_s")
            nc.tensor.matmul(hp[:], w1t[:], xr, start=True, stop=True)
            ha = work.tile([96, RPB], BF16, tag="ha")
            nc.scalar.activation(ha[0:64, :], hp[:], ACTF.Relu, bias=b1[:])
            nc.gpsimd.memset(ha[64:96, :], 1.0)

            # biased logit differences (D01, D02, D12), regions in partitions
            lp = ps_s.tile([RPB, 3], F32, tag="ps_s")
            nc.tensor.matmul(lp[:], ha[:], w2d[:], start=True, stop=True)
            cc = work.tile([RPB, 3], F32, tag="cc")
            nc.vector.tensor_scalar(cc[:], lp[:], 0.0, None, ALU.is_ge)
            # one-hot masks (first-max): m0 = c0*c1, m1 = (1-c0)*c2 stored
            # negated as (c0-1)*c2, m2 = (1-c1)*(1-c2)
            ma3 = work.tile([RPB, 3], BF16, tag="ma3")
            ccm = work.tile([RPB, 2], F32, tag="ccm")
            nc.vector.tensor_tensor(ma3[:, 0:1], cc[:, 0:1], cc[:, 1:2],
                                    ALU.mult)
            nc.vector.scalar_tensor_tensor(ma3[:, 1:2], cc[:, 0:1], 1.0,
                                           cc[:, 2:3], ALU.subtract, ALU.mult)
            nc.vector.tensor_scalar(ccm[:], cc[:, 1:3], 1.0, None, ALU.subtract)
            nc.vector.tensor_tensor(ma3[:, 2:3], ccm[:, 0:1], ccm[:, 1:2],
                                    ALU.mult)
            ma3r = work.tile([RPB, 96], BF16, tag="ma3r")
            nc.vector.tensor_copy(
                ma3r[:, :].rearrange("p (e j) -> p e j", e=3),
                ma3[:, :].broadcast_to([RPB, 3, 32]))
            return ma3r, xr

        def stage_b(b, ma3r, xr):
            # mask broadcast, masked embedding, PE injection, evict, store
            mb = ps_s.tile([96, RPB], F32, tag="ps_s")
            nc.tensor.matmul(mb[:], ma3r[:], ident[:], start=True, stop=True)
            xmm = work.tile([128, RPB], BF16, tag="xmm")
            nc.vector.tensor_tensor(xmm[0:96, :], xr, mb[:], ALU.mult)
            nc.scalar.dma_start(xmm[96:128, :], sel_d[:, :])

            ot = outp.tile([RPB, TGT * D], F32, tag="ot")
            for t in range(TGT):
                ps = ps_e.tile([RPB, D], F32, tag="ps")
                sl = slice(D * t, D * (t + 1))
                nc.tensor.matmul(ps[:], xmm[:], wall[:, sl],
                                 start=True, stop=True)
                if t < 2:
                    nc.vector.tensor_copy(ot[:, sl], ps[:])
                else:
                    nc.scalar.copy(ot[:, sl], ps[:])

            nc.sync.dma_start(ov[RPB * b:RPB * (b + 1), :], ot[:])

        # software pipeline: emit block b+1's classifier/masks before block
        # b's embedding stage so the DVE mask chain overlaps the PE burst
        prev = stage_a(0)
        for b in range(blocks):
            nxt = stage_a(b + 1) if b + 1 < blocks else None
            stage_b(b, *prev)
            prev = nxt

    nc.compile()
    return nc


def _build_wall(w_emb0, w_emb1, w_emb2):
    pe = _sinusoidal_pe(R * TGT, D)       # [128, 512] fp32
    wall = np.zeros((128, TGT * D), np.float32)
    for t in range(TGT):
        sl = slice(D * t, D * (t + 1))
        wall[8 * t:8 * t + 8, sl] = w_emb0.T
        p1 = E1_PATCH[t]
        wall[32 + 16 * p1:48 + 16 * p1, sl] = -w_emb1.T   # m1 stored negated
        wall[64:96, sl] = w_emb2.T
        # selector rows: wall[96+k, 512t+d] = pe[4k+t, d]
        wall[96:128, sl] = pe[t::TGT, :]
    return wall.astype(NPBF)


def _host_inputs(w1, b1, w2, b2, w_emb0, w_emb1, w_emb2):
    w2d = np.zeros((96, 3), np.float32)
    w2d[0:64, 0] = w2[0] - w2[1]
    w2d[0:64, 1] = w2[0] - w2[2]
    w2d[0:64, 2] = w2[1] - w2[2]
    # the ones-rows of ha are 32 wide; spread the bias over them
    w2d[64, 0] = b2[0] - b2[1]
    w2d[64, 1] = b2[0] - b2[2]
    w2d[64, 2] = b2[1] - b2[2]
    w1t96 = np.zeros((96, 64), np.float32)
    w1t96[0:32] = w1.T
    sel = np.zeros((32, RPB), np.float32)
    sel[np.arange(RPB) % 32, np.arange(RPB)] = 1.0
    return {
        "w1t": w1t96.astype(NPBF),
        "b1c": np.ascontiguousarray(b1.reshape(64, 1)),
        "w2d": np.ascontiguousarray(w2d).astype(NPBF),
        "ident": np.eye(128, dtype=np.float32).astype(NPBF),
        "sel32": sel.astype(NPBF),
        "wall": _build_wall(w_emb0, w_emb1, w_emb2),
    }


_NC_CACHE = {}


def kernel(x, w1, b1, w2, b2, w_emb0, w_emb1, w_emb2, **run_kw):
    x = np.ascontiguousarray(np.asarray(x, np.float32)).reshape(N, L)
    shared = _host_inputs(
        np.asarray(w1, np.float32), np.asarray(b1, np.float32),
        np.asarray(w2, np.float32), np.asarray(b2, np.float32),
        np.asarray(w_emb0, np.float32), np.asarray(w_emb1, np.float32),
        np.asarray(w_emb2, np.float32))

    if "nc" not in _NC_CACHE:
        _NC_CACHE["nc"] = build()
    nc = _NC_CACHE["nc"]

    in_maps = []
    for i in range(NCORES):
        m = dict(shared)
        shard = x[i * ROWS:(i + 1) * ROWS]
        m["xt"] = np.ascontiguousarray(
            shard.reshape(ROWS * R, 32).T).astype(NPBF)
        in_maps.append(m)

    res = run_bass_kernel_spmd(nc, in_maps, list(range(NCORES)), **run_kw)
    full = np.concatenate([res.results[i]["out"] for i in range(NCORES)],
                          axis=0)
    kernel.last_result = res
    return full, C


if __name__ == "__main__":
    print("smoke build only")
    build(rows=8)
    print("ok")


# revision 30
# speedup vs baseline: 6.6104x; 1.0685x over previous
"""AdaptivePatchEmbedding (MoE routing) Trainium2 Bass kernel.

Full inputs in, full output out. Shards the flattened B*C=1344 row axis
across 8 NeuronCores (168 rows each); weights are replicated. The host
pre-transposes each core's x shard to xt [32 values, rows*32 regions] in
bf16 (fp32 matmuls stream at half rate and the PE clock is capped at
1.2 GHz here, so columns are precious; PSUM still accumulates fp32).
Per core the kernel processes 42 blocks of 128 regions:
  - xm [128, 128]: rows 0:96 = x replicated per expert (DMA x3), rows
    96:128 = a one-hot region-selector constant
  - classifier h = relu(w1 @ x + b1) via zero-padded w1 (full-tile
    operands only: matmuls with partition-base > 0 operands crash the
    runtime); logit differences D01/D02 via one N=2 matmul
  - first-max argmax -> one-hot masks ma3 [128reg, 3] on DVE (m1 stored
    negated, expert-1 weights negated to compensate), replicated to
    ma3r [128, 96] by a 0-stride DVE copy, transposed+broadcast to
    mb [96, 128] by one matmul against the identity
  - xmm: masked x (rows 0:96) + selector rows untouched; ONE K=128
    matmul per output slot then computes all three experts' masked
    embeddings AND adds the positional encoding (stored in the selector
    rows of the weight matrix) in a single pass: 4x512 PE columns/block
  - PSUM evicted by plain scalar-engine copies; contiguous 1 MB DMA out
"""

import math
import sys
from contextlib import ExitStack

import numpy as np

for _p in ("/opt/trn_rl_repo",):
    if _p not in sys.path:
        sys.path.insert(0, _p)

import ml_dtypes

import concourse.bass as bass
import concourse.tile as tile
from concourse import bacc, mybir
from concourse.bass_utils import run_bass_kernel_spmd

F32 = mybir.dt.float32
BF16 = mybir.dt.bfloat16
NPBF = ml_dtypes.bfloat16
ALU = mybir.AluOpType
ACTF = mybir.ActivationFunctionType

B, C, L = 64, 21, 1024
N = B * C              # 1344 rows
NCORES = 8
ROWS = N // NCORES     # 168 rows per core
R = 32                 # regions per row (L / 32)
TGT = 4                # output slots per region
D = 512                # d_model
RPB = 128              # regions per block
E1_PATCH = [0, 0, 0, 1]   # repeat_interleave(3)[:4] for n_p=2


def _sinusoidal_pe(T, DM):
    pe = np.zeros((T, DM), np.float32)
    pos = np.arange(T, dtype=np.float32)[:, None]
    div = np.exp(np.arange(0, DM, 2, dtype=np.float32) * -(math.log(10000.0) / DM))
    pe[:, 0::2] = np.sin(pos * div)
    pe[:, 1::2] = np.cos(pos * div)
    return pe


def build(rows=ROWS):
    blocks = rows * R // RPB
    nregions = rows * R
    nc = bacc.Bacc(None, target_bir_lowering=False)

    xt_d = nc.declare_dram_parameter("xt", [32, nregions], BF16, isOutput=False)
    w1t_d = nc.declare_dram_parameter("w1t", [96, 64], BF16, isOutput=False)
    b1_d = nc.declare_dram_parameter("b1c", [64, 1], F32, isOutput=False)
    w2d_d = nc.declare_dram_parameter("w2d", [96, 3], BF16, isOutput=False)
    id_d = nc.declare_dram_parameter("ident", [128, 128], BF16, isOutput=False)
    sel_d = nc.declare_dram_parameter("sel32", [32, RPB], BF16, isOutput=False)
    # rows 0:96: expert weights zero-padded to K=32 stacked per expert
    # (expert 1 negated); rows 96:128: positional encoding routed by the
    # one-hot selector rows of xmm. Free block 512*t serves output slot t.
    wall_d = nc.declare_dram_parameter("wall", [128, TGT * D], BF16,
                                       isOutput=False)
    out_d = nc.declare_dram_parameter("out", [rows, R * TGT, D], F32,
                                      isOutput=True)

    with tile.TileContext(nc) as tc, ExitStack() as ctx:
        const = ctx.enter_context(tc.tile_pool(name="const", bufs=1))
        xin = ctx.enter_context(tc.tile_pool(name="xin", bufs=4))
        work = ctx.enter_context(tc.tile_pool(name="work", bufs=4))
        outp = ctx.enter_context(tc.tile_pool(name="outp", bufs=5))
        ps_s = ctx.enter_context(
            tc.tile_pool(name="ps_s", bufs=4, space=bass.MemorySpace.PSUM))
        ps_e = ctx.enter_context(
            tc.tile_pool(name="ps_e", bufs=4, space=bass.MemorySpace.PSUM))

        w1t = const.tile([96, 64], BF16)
        nc.sync.dma_start(w1t[:], w1t_d[:, :])
        b1 = const.tile([64, 1], F32)
        nc.sync.dma_start(b1[:], b1_d[:, :])
        w2d = const.tile([96, 3], BF16)
        nc.sync.dma_start(w2d[:], w2d_d[:, :])
        ident = const.tile([128, 128], BF16)
        nc.sync.dma_start(ident[:], id_d[:, :])
        wall = const.tile([128, TGT * D], BF16)
        nc.sync.dma_start(wall[:], wall_d[:, :])

        ov = out_d[:, :, :].rearrange("n (r t) d -> (n r) (t d)", t=TGT)

        GRP = 6
        state = {}

        def stage_a(b):
            # classifier + routing masks for block b -> (ma3r, xr)
            if b % GRP == 0:
                gw = RPB * min(GRP, blocks - b)
                xrg = xin.tile([96, RPB * GRP], BF16, tag="xrg")
                for e in range(3):
                    nc.scalar.dma_start(xrg[32 * e:32 * (e + 1), 0:gw],
                                        xt_d[:, RPB * b:RPB * b + gw])
                state["xrg"] = xrg
            xr = state["xrg"][:, RPB * (b % GRP):RPB * (b % GRP + 1)]

            # h = relu(w1 @ x + b1); rows 64:96 all-ones so the augmented
            # w2d rows add the b2 bias
            hp = ps_s.tile([64, RPB], F32, tag="ps_s")
            nc.tensor.matmul(hp[:], w1t[:], xr, start=True, stop=True)
            ha = work.tile([96, RPB], BF16, tag="ha")
            nc.scalar.activation(ha[0:64, :], hp[:], ACTF.Relu, bias=b1[:])
            nc.gpsimd.memset(ha[64:96, :], 1.0)

            # biased logit differences (D01, D02, D12), regions in partitions
            lp = ps_s.tile([RPB, 3], F32, tag="ps_s")
            nc.tensor.matmul(lp[:], ha[:], w2d[:], start=True, stop=True)
            cc = work.tile([RPB, 3], F32, tag="cc")
            nc.vector.tensor_scalar(cc[:], lp[:], 0.0, None, ALU.is_ge)
            # one-hot masks (first-max): m0 = c0*c1, m1 = (1-c0)*c2 stored
            # negated as (c0-1)*c2, m2 = (1-c1)*(1-c2)
            ma3 = work.tile([RPB, 3], BF16, tag="ma3")
            ccm = work.tile([RPB, 2], F32, tag="ccm")
            nc.vector.tensor_tensor(ma3[:, 0:1], cc[:, 0:1], cc[:, 1:2],
                                    ALU.mult)
            nc.vector.scalar_tensor_tensor(ma3[:, 1:2], cc[:, 0:1], 1.0,
                                           cc[:, 2:3], ALU.subtract, ALU.mult)
            nc.vector.tensor_scalar(ccm[:], cc[:, 1:3], 1.0, None, ALU.subtract)
            nc.vector.tensor_tensor(ma3[:, 2:3], ccm[:, 0:1], ccm[:, 1:2],
                                    ALU.mult)
            ma3r = work.tile([RPB, 96], BF16, tag="ma3r")
            nc.vector.tensor_copy(
                ma3r[:, :].rearrange("p (e j) -> p e j", e=3),
                ma3[:, :].broadcast_to([RPB, 3, 32]))
            # prefetch the selector rows into the next xmm a block early
            xmm = work.tile([128, RPB], BF16, tag="xmm")
            nc.sync.dma_start(xmm[96:128, :], sel_d[:, :])
            return ma3r, xr, xmm

        def stage_b(b, ma3r, xr, xmm):
            # mask broadcast, masked embedding, PE injection, evict, store
            mb = ps_s.tile([96, RPB], F32, tag="ps_s")
            nc.tensor.matmul(mb[:], ma3r[:], ident[:], start=True, stop=True)
            nc.vector.tensor_tensor(xmm[0:96, :], xr, mb[:], ALU.mult)

            ot = outp.tile([RPB, TGT * D], F32, tag="ot")
            for t in range(TGT):
                ps = ps_e.tile([RPB, D], F32, tag="ps")
                sl = slice(D * t, D * (t + 1))
                nc.tensor.matmul(ps[:], xmm[:], wall[:, sl],
                                 start=True, stop=True)
                if t < 2:
                    nc.vector.tensor_copy(ot[:, sl], ps[:])
                else:
                    nc.scalar.copy(ot[:, sl], ps[:])

            eng = nc.sync if b % 2 == 0 else nc.scalar
            eng.dma_start(ov[RPB * b:RPB * (b + 1), :], ot[:])

        # software pipeline: emit block b+1's classifier/masks before block
        # b's embedding stage so the DVE mask chain overlaps the PE burst
        prev = stage_a(0)
        for b in range(blocks):
            nxt = stage_a(b + 1) if b + 1 < blocks else None
            stage_b(b, *prev)
            prev = nxt

    nc.compile()
    return nc


def _build_wall(w_emb0, w_emb1, w_emb2):
    pe = _sinusoidal_pe(R * TGT, D)       # [128, 512] fp32
    wall = np.zeros((128, TGT * D), np.float32)
    for t in range(TGT):
        sl = slice(D * t, D * (t + 1))
        wall[8 * t:8 * t + 8, sl] = w_emb0.T
        p1 = E1_PATCH[t]
        wall[32 + 16 * p1:48 + 16 * p1, sl] = -w_emb1.T   # m1 stored negated
        wall[64:96, sl] = w_emb2.T
        # selector rows: wall[96+k, 512t+d] = pe[4k+t, d]
        wall[96:128, sl] = pe[t::TGT, :]
    return wall.astype(NPBF)


def _host_inputs(w1, b1, w2, b2, w_emb0, w_emb1, w_emb2):
    w2d = np.zeros((96, 3), np.float32)
    w2d[0:64, 0] = w2[0] - w2[1]
    w2d[0:64, 1] = w2[0] - w2[2]
    w2d[0:64, 2] = w2[1] - w2[2]
    # the ones-rows of ha are 32 wide; spread the bias over them
    w2d[64, 0] = b2[0] - b2[1]
    w2d[64, 1] = b2[0] - b2[2]
    w2d[64, 2] = b2[1] - b2[2]
    w1t96 = np.zeros((96, 64), np.float32)
    w1t96[0:32] = w1.T
    sel = np.zeros((32, RPB), np.float32)
    sel[np.arange(RPB) % 32, np.arange(RPB)] = 1.0
    return {
        "w1t": w1t96.astype(NPBF),
        "b1c": np.ascontiguousarray(b1.reshape(64, 1)),
        "w2d": np.ascontiguousarray(w2d).astype(NPBF),
        "ident": np.eye(128, dtype=np.float32).astype(NPBF),
        "sel32": sel.astype(NPBF),
        "wall": _build_wall(w_emb0, w_emb1, w_emb2),
    }


_NC_CACHE = {}


def kernel(x, w1, b1, w2, b2, w_emb0, w_emb1, w_emb2, **run_kw):
    x = np.ascontiguousarray(np.asarray(x, np.float32)).reshape(N, L)
    shared = _host_inputs(
        np.asarray(w1, np.float32), np.asarray(b1, np.float32),
        np.asarray(w2, np.float32), np.asarray(b2, np.float32),
        np.asarray(w_emb0, np.float32), np.asarray(w_emb1, np.float32),
        np.asarray(w_emb2, np.float32))

    if "nc" not in _NC_CACHE:
        _NC_CACHE["nc"] = build()
    nc = _NC_CACHE["nc"]

    in_maps = []
    for i in range(NCORES):
        m = dict(shared)
        shard = x[i * ROWS:(i + 1) * ROWS]
        m["xt"] = np.ascontiguousarray(
            shard.reshape(ROWS * R, 32).T).astype(NPBF)
        in_maps.append(m)

    res = run_bass_kernel_spmd(nc, in_maps, list(range(NCORES)), **run_kw)
    full = np.concatenate([res.results[i]["out"] for i in range(NCORES)],
                          axis=0)
    kernel.last_result = res
    return full, C


if __name__ == "__main__":
    print("smoke build only")
    build(rows=8)
    print("ok")


# revision 32
# speedup vs baseline: 6.6696x; 1.0089x over previous
"""AdaptivePatchEmbedding (MoE routing) Trainium2 Bass kernel.

Full inputs in, full output out. Shards the flattened B*C=1344 row axis
across 8 NeuronCores (168 rows each); weights are replicated. The host
pre-transposes each core's x shard to xt [32 values, rows*32 regions] in
bf16 (fp32 matmuls stream at half rate and the PE clock is capped at
1.2 GHz here, so columns are precious; PSUM still accumulates fp32).
Per core the kernel processes 42 blocks of 128 regions:
  - xm [128, 128]: rows 0:96 = x replicated per expert (DMA x3), rows
    96:128 = a one-hot region-selector constant
  - classifier h = relu(w1 @ x + b1) via zero-padded w1 (full-tile
    operands only: matmuls with partition-base > 0 operands crash the
    runtime); logit differences D01/D02 via one N=2 matmul
  - first-max argmax -> one-hot masks ma3 [128reg, 3] on DVE (m1 stored
    negated, expert-1 weights negated to compensate), replicated to
    ma3r [128, 96] by a 0-stride DVE copy, transposed+broadcast to
    mb [96, 128] by one matmul against the identity
  - xmm: masked x (rows 0:96) + selector rows untouched; ONE K=128
    matmul per output slot then computes all three experts' masked
    embeddings AND adds the positional encoding (stored in the selector
    rows of the weight matrix) in a single pass: 4x512 PE columns/block
  - PSUM evicted by plain scalar-engine copies; contiguous 1 MB DMA out
"""

import math
import sys
from contextlib import ExitStack

import numpy as np

for _p in ("/opt/trn_rl_repo",):
    if _p not in sys.path:
        sys.path.insert(0, _p)

import ml_dtypes

import concourse.bass as bass
import concourse.tile as tile
from concourse import bacc, mybir
from concourse.bass_utils import run_bass_kernel_spmd

F32 = mybir.dt.float32
BF16 = mybir.dt.bfloat16
NPBF = ml_dtypes.bfloat16
ALU = mybir.AluOpType
ACTF = mybir.ActivationFunctionType

B, C, L = 64, 21, 1024
N = B * C              # 1344 rows
NCORES = 8
ROWS = N // NCORES     # 168 rows per core
R = 32                 # regions per row (L / 32)
TGT = 4                # output slots per region
D = 512                # d_model
RPB = 128              # regions per block
E1_PATCH = [0, 0, 0, 1]   # repeat_interleave(3)[:4] for n_p=2


def _sinusoidal_pe(T, DM):
    pe = np.zeros((T, DM), np.float32)
    pos = np.arange(T, dtype=np.float32)[:, None]
    div = np.exp(np.arange(0, DM, 2, dtype=np.float32) * -(math.log(10000.0) / DM))
    pe[:, 0::2] = np.sin(pos * div)
    pe[:, 1::2] = np.cos(pos * div)
    return pe


def build(rows=ROWS):
    blocks = rows * R // RPB
    nregions = rows * R
    nc = bacc.Bacc(None, target_bir_lowering=False)

    xt_d = nc.declare_dram_parameter("xt", [32, nregions], BF16, isOutput=False)
    w1t_d = nc.declare_dram_parameter("w1t", [96, 64], BF16, isOutput=False)
    b1_d = nc.declare_dram_parameter("b1c", [64, 1], F32, isOutput=False)
    w2d_d = nc.declare_dram_parameter("w2d", [96, 3], BF16, isOutput=False)
    id_d = nc.declare_dram_parameter("ident", [128, 128], BF16, isOutput=False)
    sel_d = nc.declare_dram_parameter("sel32", [32, RPB], BF16, isOutput=False)
    # rows 0:96: expert weights zero-padded to K=32 stacked per expert
    # (expert 1 negated); rows 96:128: positional encoding routed by the
    # one-hot selector rows of xmm. Free block 512*t serves output slot t.
    wall_d = nc.declare_dram_parameter("wall", [128, TGT * D], BF16,
                                       isOutput=False)
    out_d = nc.declare_dram_parameter("out", [rows, R * TGT, D], F32,
                                      isOutput=True)

    with tile.TileContext(nc) as tc, ExitStack() as ctx:
        const = ctx.enter_context(tc.tile_pool(name="const", bufs=1))
        xin = ctx.enter_context(tc.tile_pool(name="xin", bufs=4))
        work = ctx.enter_context(tc.tile_pool(name="work", bufs=4))
        outp = ctx.enter_context(tc.tile_pool(name="outp", bufs=5))
        ps_s = ctx.enter_context(
            tc.tile_pool(name="ps_s", bufs=4, space=bass.MemorySpace.PSUM))
        ps_e = ctx.enter_context(
            tc.tile_pool(name="ps_e", bufs=4, space=bass.MemorySpace.PSUM))

        w1t = const.tile([96, 64], BF16)
        nc.sync.dma_start(w1t[:], w1t_d[:, :])
        b1 = const.tile([64, 1], F32)
        nc.sync.dma_start(b1[:], b1_d[:, :])
        w2d = const.tile([96, 3], BF16)
        nc.sync.dma_start(w2d[:], w2d_d[:, :])
        ident = const.tile([128, 128], BF16)
        nc.sync.dma_start(ident[:], id_d[:, :])
        wall = const.tile([128, TGT * D], BF16)
        nc.sync.dma_start(wall[:], wall_d[:, :])

        ov = out_d[:, :, :].rearrange("n (r t) d -> (n r) (t d)", t=TGT)

        # four fixed xmm buffers; selector rows written once, reused forever
        xmm_bank = []
        for i in range(4):
            xmm_slot = const.tile([128, RPB], BF16, tag=f"xmm{i}",
                                  name=f"xmm{i}")
            xmm_bank.append(xmm_slot)
        for t_ in xmm_bank:
            nc.sync.dma_start(t_[96:128, :], sel_d[:, :])

        GRP = 6
        state = {}

        def stage_a(b):
            # classifier + routing masks for block b -> (ma3r, xr)
            if b % GRP == 0:
                gw = RPB * min(GRP, blocks - b)
                xrg = xin.tile([96, RPB * GRP], BF16, tag="xrg")
                for e in range(3):
                    nc.scalar.dma_start(xrg[32 * e:32 * (e + 1), 0:gw],
                                        xt_d[:, RPB * b:RPB * b + gw])
                state["xrg"] = xrg
            xr = state["xrg"][:, RPB * (b % GRP):RPB * (b % GRP + 1)]

            # h = relu(w1 @ x + b1); rows 64:96 all-ones so the augmented
            # w2d rows add the b2 bias
            hp = ps_s.tile([64, RPB], F32, tag="ps_s")
            nc.tensor.matmul(hp[:], w1t[:], xr, start=True, stop=True)
            ha = work.tile([96, RPB], BF16, tag="ha")
            nc.scalar.activation(ha[0:64, :], hp[:], ACTF.Relu, bias=b1[:])
            nc.gpsimd.memset(ha[64:96, :], 1.0)

            # biased logit differences (D01, D02, D12), regions in partitions
            lp = ps_s.tile([RPB, 3], F32, tag="ps_s")
            nc.tensor.matmul(lp[:], ha[:], w2d[:], start=True, stop=True)
            cc = work.tile([RPB, 3], F32, tag="cc")
            nc.vector.tensor_scalar(cc[:], lp[:], 0.0, None, ALU.is_ge)
            # one-hot masks (first-max): m0 = c0*c1, m1 = (1-c0)*c2 stored
            # negated as (c0-1)*c2, m2 = (1-c1)*(1-c2)
            ma3 = work.tile([RPB, 3], BF16, tag="ma3")
            ccm = work.tile([RPB, 2], F32, tag="ccm")
            nc.vector.tensor_tensor(ma3[:, 0:1], cc[:, 0:1], cc[:, 1:2],
                                    ALU.mult)
            nc.vector.scalar_tensor_tensor(ma3[:, 1:2], cc[:, 0:1], 1.0,
                                           cc[:, 2:3], ALU.subtract, ALU.mult)
            nc.vector.tensor_scalar(ccm[:], cc[:, 1:3], 1.0, None, ALU.subtract)
            nc.vector.tensor_tensor(ma3[:, 2:3], ccm[:, 0:1], ccm[:, 1:2],
                                    ALU.mult)
            ma3r = work.tile([RPB, 96], BF16, tag="ma3r")
            nc.vector.tensor_copy(
                ma3r[:, :].rearrange("p (e j) -> p e j", e=3),
                ma3[:, :].broadcast_to([RPB, 3, 32]))
            return ma3r, xr

        def stage_b(b, ma3r, xr):
            # mask broadcast, masked embedding, PE injection, evict, store
            mb = ps_s.tile([96, RPB], F32, tag="ps_s")
            nc.tensor.matmul(mb[:], ma3r[:], ident[:], start=True, stop=True)
            xmm = xmm_bank[b % 4]
            nc.vector.tensor_tensor(xmm[0:96, :], xr, mb[:], ALU.mult)

            ot = outp.tile([RPB, TGT * D], F32, tag="ot")
            for t in range(TGT):
                ps = ps_e.tile([RPB, D], F32, tag="ps")
                sl = slice(D * t, D * (t + 1))
                nc.tensor.matmul(ps[:], xmm[:], wall[:, sl],
                                 start=True, stop=True)
                if t < 2:
                    nc.vector.tensor_copy(ot[:, sl], ps[:])
                else:
                    nc.scalar.copy(ot[:, sl], ps[:])

            eng = nc.sync if b % 2 == 0 else nc.scalar
            eng.dma_start(ov[RPB * b:RPB * (b + 1), :], ot[:])

        # software pipeline: emit block b+1's classifier/masks before block
        # b's embedding stage so the DVE mask chain overlaps the PE burst
        prev = stage_a(0)
        for b in range(blocks):
            nxt = stage_a(b + 1) if b + 1 < blocks else None
            stage_b(b, *prev)
            prev = nxt

    nc.compile()
    return nc


def _build_wall(w_emb0, w_emb1, w_emb2):
    pe = _sinusoidal_pe(R * TGT, D)       # [128, 512] fp32
    wall = np.zeros((128, TGT * D), np.float32)
    for t in range(TGT):
        sl = slice(D * t, D * (t + 1))
        wall[8 * t:8 * t + 8, sl] = w_emb0.T
        p1 = E1_PATCH[t]
        wall[32 + 16 * p1:48 + 16 * p1, sl] = -w_emb1.T   # m1 stored negated
        wall[64:96, sl] = w_emb2.T
        # selector rows: wall[96+k, 512t+d] = pe[4k+t, d]
        wall[96:128, sl] = pe[t::TGT, :]
    return wall.astype(NPBF)


def _host_inputs(w1, b1, w2, b2, w_emb0, w_emb1, w_emb2):
    w2d = np.zeros((96, 3), np.float32)
    w2d[0:64, 0] = w2[0] - w2[1]
    w2d[0:64, 1] = w2[0] - w2[2]
    w2d[0:64, 2] = w2[1] - w2[2]
    # the ones-rows of ha are 32 wide; spread the bias over them
    w2d[64, 0] = b2[0] - b2[1]
    w2d[64, 1] = b2[0] - b2[2]
    w2d[64, 2] = b2[1] - b2[2]
    w1t96 = np.zeros((96, 64), np.float32)
    w1t96[0:32] = w1.T
    sel = np.zeros((32, RPB), np.float32)
    sel[np.arange(RPB) % 32, np.arange(RPB)] = 1.0
    return {
        "w1t": w1t96.astype(NPBF),
        "b1c": np.ascontiguousarray(b1.reshape(64, 1)),
        "w2d": np.ascontiguousarray(w2d).astype(NPBF),
        "ident": np.eye(128, dtype=np.float32).astype(NPBF),
        "sel32": sel.astype(NPBF),
        "wall": _build_wall(w_emb0, w_emb1, w_emb2),
    }


_NC_CACHE = {}


def kernel(x, w1, b1, w2, b2, w_emb0, w_emb1, w_emb2, **run_kw):
    x = np.ascontiguousarray(np.asarray(x, np.float32)).reshape(N, L)
    shared = _host_inputs(
        np.asarray(w1, np.float32), np.asarray(b1, np.float32),
        np.asarray(w2, np.float32), np.asarray(b2, np.float32),
        np.asarray(w_emb0, np.float32), np.asarray(w_emb1, np.float32),
        np.asarray(w_emb2, np.float32))

    if "nc" not in _NC_CACHE:
        _NC_CACHE["nc"] = build()
    nc = _NC_CACHE["nc"]

    in_maps = []
    for i in range(NCORES):
        m = dict(shared)
        shard = x[i * ROWS:(i + 1) * ROWS]
        m["xt"] = np.ascontiguousarray(
            shard.reshape(ROWS * R, 32).T).astype(NPBF)
        in_maps.append(m)

    res = run_bass_kernel_spmd(nc, in_maps, list(range(NCORES)), **run_kw)
    full = np.concatenate([res.results[i]["out"] for i in range(NCORES)],
                          axis=0)
    kernel.last_result = res
    return full, C


if __name__ == "__main__":
    print("smoke build only")
    build(rows=8)
    print("ok")
